# revision 49
# baseline (speedup 1.0000x reference)
"""Trainium2 Bass kernel for nn_Bert_Proj_CRF (embed -> proj -> MLP -> CRF loss).

Data-parallel over batch across 8 NeuronCores (8 batch elements per core).

v2 design (vs v1 ucode-transpose-gather baseline):
  - Embedding rows arrive via SWDGE *indirect* DMA (std descriptor-gen path,
    no Q7 ucode library -> no ~11us library reload) into a token-major
    staging buffer, then 96 XBAR transpose DMAs ([128,128] u16 each) flip
    them into the PE feature-major layout.
  - fc1 runs as fp8 DoubleRow matmuls (K=256 per matmul), domain projection
    as a K=10 fp8 DoubleRow matmul against a host-precomputed Vd = dom_w @ W1d^T.
  - CRF runs unnormalized: M = (exp(trans)/8) * exp(pred); softmax
    denominators cancel against the gold-path emission score analytically
    (correction terms msum1*log8 and (1-m0)*logden0; gold trans/start/end
    score computed on host from target/words/trans).
  - Scan step = bf16 broadcast mul + 3 contiguous pairwise adds (faster than
    InstTensorReduce which runs at fp32 rate); Mt construction runs on the
    otherwise idle GpSimd engine.

Per-core layout: partition u = b*16 + c (b batch-in-shard, c chunk), position
s = c*32 + l. Token (l, u) lives at column l*128+u of its 4-l group tile.
"""

import sys

for _p in ("/opt/trn_rl_repo", "/root/.axon_site/_ro/trn_rl_repo"):
    if _p not in sys.path:
        sys.path.append(_p)

import numpy as np
import ml_dtypes

import concourse.bass as bass
import concourse.tile as tile
from concourse import bacc, mybir
from concourse._compat import with_exitstack

F32 = mybir.dt.float32
BF16 = mybir.dt.bfloat16
FP8 = mybir.dt.float8e4
I32 = mybir.dt.int32
I16 = mybir.dt.int16
DR = mybir.MatmulPerfMode.DoubleRow
ADD = mybir.AluOpType.add
MULT = mybir.AluOpType.mult
SUB = mybir.AluOpType.subtract

VOCAB = 21128
E = 768
H = 256
NC_N = 8
B = 64
S = 512
T = 8
NCORP = 10

BSH = B // NC_N   # 8 batch elements per core
L = 32            # chunk length
K = 16            # chunks per batch element
TOK = BSH * S     # 4096 tokens per core
FKT = 3           # fp8 K-tile pairs (256 features each)
XSC = 128.0
PSC = 1.0 / (XSC * XSC)
LOG8 = float(np.log(8.0))

# gather/compute groups: (l_start, n_l). 256-token gathers at the edges so
# the pipeline head starts fast and the tail chain after the last drain is
# short; 512-token gathers in the middle (cheapest per token).
GROUPS = [(0, 2), (2, 2), (4, 4), (8, 4), (12, 4), (16, 4), (20, 4),
          (24, 4), (28, 2), (30, 2)]
NGRP = len(GROUPS)

# bf16 blob layout: name -> width (per partition)
_BF_SPECS = [("mscb", L), ("wneg", L), ("ohcm", L * T), ("estart", T),
             ("einitb", T * T), ("eendr", T), ("estar", T * T)]
BF_OFF = {}
_o = 0
for _k, _w in _BF_SPECS:
    BF_OFF[_k] = (_o, _w)
    _o += _w
BF_W = _o

_F32_SPECS = [("f1b2", 2), ("hostc", 1), ("c0m", 1), ("ind0", 1),
              ("indinv", 1), ("g1", T)]
F32_OFF = {}
_o = 0
for _k, _w in _F32_SPECS:
    F32_OFF[_k] = (_o, _w)
    _o += _w
F32_W = _o

INPUT_KEYS = ["emb8", "idx16", "idx32", "bfb", "fb32", "f1w8", "vd8", "ocr8",
              "f2wb", "onesb", "shf8"]


def _act(name):
    return getattr(mybir.ActivationFunctionType, name)


@with_exitstack
def _build_kernel(ctx, tc, io):
    nc = tc.nc
    d = io
    ctx.enter_context(nc.allow_low_precision(
        reason="CRF exp-domain products tolerate bf16; loss tol is 2e-2"))

    pool = ctx.enter_context(tc.tile_pool(name="main", bufs=1))
    dpool = ctx.enter_context(tc.tile_pool(name="dbl", bufs=2))
    hpool = ctx.enter_context(tc.tile_pool(name="hps", bufs=3, space="PSUM"))
    ppool = ctx.enter_context(tc.tile_pool(name="pps", bufs=3, space="PSUM"))
    spool = ctx.enter_context(tc.tile_pool(name="sps", bufs=1, space="PSUM"))

    # ---- index load first; then the transposed ucode gathers ----
    # idx16: wrapped layout, one column-block per gather (i at [i%16, i//16])
    idx_sb = pool.tile([128, TOK // 16], I16, tag="idx")
    nc.sync.dma_start(idx_sb[:], d["idx16"][:])

    # xT8: per-gather blocks [kt(3), tok(GT), s(2)] fp8, contiguous stream
    xT8 = pool.tile([128, 3 * TOK * 2], FP8, tag="xT8")

    def gather_block(gi):
        l0, nl = GROUPS[gi]
        gt = nl * 128
        off = 6 * (l0 * 128)
        gv = xT8[:, off:off + 6 * gt].rearrange("p (a b) -> p a b", b=gt)
        nc.gpsimd.dma_gather(
            gv, d["emb8"][:], idx_sb[:, (l0 * 128) // 16:(l0 * 128 + gt) // 16],
            gt, gt, E, transpose=True, queue_num=gi % 4)

    # Pool queue stays gather-only ahead of the pair products (extra Pool
    # tensor ops would force ucode library switches); 4 rings pace the stream.
    for gi in range(NGRP):
        gather_block(gi)

    # ---- constant loads (SP HWDGE) ----
    def load(name, shape, dtype, src):
        t = pool.tile(shape, dtype, tag=name)
        nc.sync.dma_start(t[:], src)
        return t

    bfb = load("bfb", [128, BF_W], BF16, d["bfb"][:])
    fb32 = load("fb32", [128, F32_W], F32, d["fb32"][:])
    f1w8 = load("f1w8", [128, FKT, 2, 2, 128], FP8,
                d["f1w8"].rearrange("(a b c p) m -> p a b c m",
                                    a=FKT, b=2, c=2, p=128))
    vdb = load("vdb", [128, 2, 128], FP8,
               d["vd8"].rearrange("p (a m) -> p a m", a=2))
    ocrb = load("ocrb", [128, 512], FP8, d["ocr8"][:])
    f2w = load("f2w", [128, 2, T], BF16,
               d["f2wb"].rearrange("p (a b) -> p a b", b=T))
    onesb = load("onesb", [1, 160], BF16, d["onesb"][:])
    shf = load("shf", [128, 4, 128], BF16,
               d["shf8"].rearrange("p (a b) -> p a b", b=128))

    def bview(key, *dims):
        off, w = BF_OFF[key]
        v = bfb[:, off:off + w]
        if len(dims) == 2:
            v = v.rearrange("p (a b) -> p a b", b=dims[1])
        return v

    def fview(key):
        off, w = F32_OFF[key]
        return fb32[:, off:off + w]

    mscb = bview("mscb")            # [128, 32]
    wneg = bview("wneg")            # [128, 32]
    ohcm = bview("ohcm", L, T)      # [128, 32, 8]
    estart = bview("estart")        # [128, 8]
    einitb = bview("einitb")        # [128, 64]
    eendr = bview("eendr")          # [128, 8]
    estar = bview("estar", T, T)    # [128, 8, 8]  (k, j)
    f1b2 = fview("f1b2")

    # ---- persistent tiles ----
    h = pool.tile([128, 2, TOK], BF16, tag="h")
    esel = pool.tile([128, L, T], F32, tag="esel")
    P = pool.tile([128, T, T], BF16, tag="P")
    tmp = pool.tile([128, T, T, T], BF16, tag="tmp")
    er0 = pool.tile([128, T], BF16, tag="er0")
    Mtall = pool.tile([128, L, T, T], BF16, tag="Mtall")
    # Pool-side pair-combine workspace: Mtp[pi] = Mt[12+2pi] @ Mt[13+2pi]
    PAIR_L0 = 12
    NPAIR = (L - PAIR_L0) // 2
    Mtp = pool.tile([128, NPAIR, T, T], BF16, tag="Mtp")
    tmpP = pool.tile([128, T, T, T], BF16, tag="tmpP")
    t1P = pool.tile([128, T * T, 4], BF16, tag="t1P")
    t2P = pool.tile([128, T * T, 2], BF16, tag="t2P")

    def scan_mulred(min1):
        """P <- P @ (min1 as M^T): tmp[a,k,j] = P[a,j]*min1[k,j]; reduce j."""
        nc.vector.tensor_mul(
            tmp[:],
            P[:].unsqueeze(2).broadcast_to([128, T, T, T]),
            min1.unsqueeze(1).broadcast_to([128, T, T, T]))
        nc.vector.reduce_sum(P[:], tmp[:], axis=mybir.AxisListType.X)

    def pool_pair(l, pi):
        """Mtp[pi] <- pair product, transposed storage: Mtp[k, j] =
        sum_m Mt[l+1][k, m] * Mt[l][m, j], entirely on the Pool engine.
        tmpP dims mean [p, k, j, m]; reduce over innermost m via 3 adds."""
        nc.gpsimd.tensor_mul(
            tmpP[:],
            Mtall[:, l + 1].unsqueeze(2).broadcast_to([128, T, T, T]),
            Mtall[:, l].transpose([0, 2, 1]).unsqueeze(1)
            .broadcast_to([128, T, T, T]))
        tv = tmpP[:].rearrange("p a b c -> p (a b) c")
        nc.gpsimd.tensor_add(t1P[:], tv[:, :, 0:4], tv[:, :, 4:8])
        nc.gpsimd.tensor_add(t2P[:], t1P[:, :, 0:2], t1P[:, :, 2:4])
        nc.gpsimd.tensor_add(Mtp[:, pi].rearrange("p a k -> p (a k)"),
                             t2P[:, :, 0], t2P[:, :, 1])

    # ---- per-group pipeline ----
    for t in range(NGRP):
        l0, nl = GROUPS[t]
        gt = nl * 128
        ls = slice(l0, l0 + nl)
        # fc1: 3 fp8 DoubleRow K-tiles + Vd (K=10, fp8 DoubleRow)
        for ch in range(2):
            ph = hpool.tile([128, gt], F32, tag="ph")
            for kt in range(FKT):
                rhs = bass.AP(xT8[:].tensor,
                              xT8[:].offset + 6 * (l0 * 128) + kt * (2 * gt),
                              [list(xT8[:].ap[0]), [1, 2], [2, gt]])
                nc.tensor.matmul(ph[:], f1w8[:, kt, ch], rhs,
                                 start=(kt == 0), stop=False, perf_mode=DR)
            nc.tensor.matmul(ph[:], vdb[:, ch, :], ocrb[:, :gt],
                             start=False, stop=True)
            nc.scalar.activation(h[:, ch, l0 * 128:l0 * 128 + gt], ph[:],
                                 _act("Relu"), bias=f1b2[:, ch:ch + 1],
                                 scale=PSC)
        # fc2 (+ f2b via K=1 matmul)
        pp = ppool.tile([128, nl, T], F32, tag="pp")
        ppf = pp[:].rearrange("p a b -> p (a b)")
        nc.tensor.matmul(ppf, onesb[0:1, 0:128], onesb[0:1, 128:128 + nl * T],
                         start=True, stop=False, skip_group_check=True)
        for li in range(nl):
            for ch in range(2):
                nc.tensor.matmul(
                    pp[:, li, :],
                    h[:, ch, (l0 + li) * 128:(l0 + li + 1) * 128],
                    f2w[:, ch, :],
                    start=False, stop=(li == nl - 1 and ch == 1),
                    skip_group_check=True)
        # emissions
        etr = dpool.tile([128, nl, T], BF16, tag="etr")
        nc.scalar.activation(etr[:], pp[:], _act("Exp"))
        nc.vector.tensor_mul(esel[:, ls, :], pp[:], ohcm[:, ls, :])
        if t == 0:
            nc.scalar.activation(er0[:], etr[:, 0, :], _act("Copy"))
        # etm = etr*mscb; Mt[l',k,j] = estar[k,j]*etm[l',k]; diag += wneg
        etm = dpool.tile([128, nl, T], BF16, tag="etm")
        nc.vector.tensor_mul(
            etm[:], etr[:],
            mscb[:, ls].unsqueeze(2).broadcast_to([128, nl, T]))
        Mt = Mtall[:, ls]
        nc.vector.tensor_mul(
            Mt,
            estar[:].unsqueeze(1).broadcast_to([128, nl, T, T]),
            etm[:].unsqueeze(3).broadcast_to([128, nl, T, T]))
        diag = bass.AP(Mt.tensor, Mt.offset,
                       [list(Mt.ap[0]), [T + 1, T], [T * T, nl]])
        nc.vector.tensor_add(
            diag, diag,
            wneg[:, ls].unsqueeze(1).broadcast_to([128, T, nl]))
        # P init (group 0): P[a, j] = etr[0, j]*estart[j]  (+ I off c0 rows)
        if t == 0:
            nc.vector.tensor_mul(
                P[:],
                etr[:, 0, :].unsqueeze(1).broadcast_to([128, T, T]),
                estart[:].unsqueeze(1).broadcast_to([128, T, T]))
            nc.vector.tensor_add(P[:].rearrange("p a b -> p (a b)"),
                                 P[:].rearrange("p a b -> p (a b)"),
                                 einitb[:])
        # serial scan steps: per-l below PAIR_L0, pair-combined (Pool) above
        if l0 < PAIR_L0:
            for li in range(nl):
                scan_mulred(Mtall[:, l0 + li])
        else:
            for pi in range((l0 - PAIR_L0) // 2,
                            (l0 - PAIR_L0) // 2 + nl // 2):
                pool_pair(PAIR_L0 + 2 * pi, pi)
                scan_mulred(Mtp[:, pi])

    # ---- cross-chunk combine: P[p] <- P[p] @ P[p+off], off = 1,2,4;
    # the off=8 level only matters on the c==0 rows, where all P rows are
    # equal, so it collapses to a row-vector x matrix product. ----
    for si in range(3):
        bcp = spool.tile([128, T * T], F32, tag="bcp")
        # rhs[p, (k j)] = P[p, j, k]  (transposed view)
        rhsT = bass.AP(P[:].tensor, P[:].offset,
                       [list(P[:].ap[0]), [1, T], [T, T]])
        nc.tensor.matmul(bcp[:], shf[:, si, :], rhsT,
                         start=True, stop=True)
        scan_mulred(bcp[:].rearrange("p (a b) -> p a b", b=T))
    bcp8 = spool.tile([128, T * T], F32, tag="bcp")
    rhsT = bass.AP(P[:].tensor, P[:].offset,
                   [list(P[:].ap[0]), [1, T], [T, T]])
    nc.tensor.matmul(bcp8[:], shf[:, 3, :], rhsT, start=True, stop=True)
    # row 0 only: v0[k] = sum_j P[0, j] * bcp8[k, j]
    tmp4 = pool.tile([128, T, T], BF16, tag="tmp4")
    nc.vector.tensor_mul(
        tmp4[:],
        P[:, 0, :].unsqueeze(1).broadcast_to([128, T, T]),
        bcp8[:].rearrange("p (a b) -> p a b", b=T))
    v0 = pool.tile([128, T], F32, tag="v0")
    nc.vector.reduce_sum(v0[:], tmp4[:], axis=mybir.AxisListType.X)

    # ---- finals ----
    nmv = pool.tile([128, T], BF16, tag="nmv")
    nc.vector.tensor_mul(nmv[:], v0[:], eendr[:])
    ns = pool.tile([128, 2], F32, tag="ns")
    nc.vector.reduce_sum(ns[:, 0:1], nmv[:], axis=mybir.AxisListType.X)
    nc.vector.reduce_sum(ns[:, 1:2], er0[:], axis=mybir.AxisListType.X)
    # mask off non-c0 rows to 1.0 before Ln (avoid Ln of junk)
    ns2 = pool.tile([128, 2], F32, tag="ns2")
    nc.vector.scalar_tensor_tensor(
        ns2[:], ns[:], fview("ind0"),
        fview("indinv").broadcast_to([128, 2]), MULT, ADD)
    lnv = pool.tile([128, 2], F32, tag="lnv")
    nc.scalar.activation(lnv[:], ns2[:], _act("Ln"))
    esum = pool.tile([128, 1], F32, tag="esum")
    nc.vector.reduce_sum(esum[:], esel[:], axis=mybir.AxisListType.XY)
    # v = (normv + hostc) - (logden0*c0m + esum)
    w1 = pool.tile([128, 1], F32, tag="w1")
    nc.vector.scalar_tensor_tensor(w1[:], lnv[:, 1:2], fview("c0m"),
                                   esum[:], MULT, ADD)
    v = pool.tile([128, 1], F32, tag="v")
    nc.vector.scalar_tensor_tensor(v[:], lnv[:, 0:1], fview("hostc"),
                                   w1[:], ADD, SUB)
    lps = spool.tile([T, 1], F32, tag="lps")
    nc.tensor.matmul(lps[:], fview("g1"), v[:], start=True, stop=True)
    loss = pool.tile([T, 1], F32, tag="loss")
    nc.scalar.activation(loss[:], lps[:], _act("Copy"))
    nc.sync.dma_start(io["loss8"][:], loss[:])


def _declare_io(nc):
    d = {}

    def inp(name, shape, dtype):
        d[name] = nc.dram_tensor(name, shape, dtype, kind="ExternalInput").ap()

    inp("emb8", [VOCAB, E], FP8)
    inp("idx16", [128, TOK // 16], I16)
    inp("idx32", [128, 4], I32)
    inp("bfb", [128, BF_W], BF16)
    inp("fb32", [128, F32_W], F32)
    inp("f1w8", [FKT * 2 * 2 * 128, 128], FP8)
    inp("vd8", [128, 256], FP8)
    inp("ocr8", [128, 512], FP8)
    inp("f2wb", [128, 2 * T], BF16)
    inp("onesb", [1, 160], BF16)
    inp("shf8", [128, 4 * 128], BF16)
    d["loss8"] = nc.dram_tensor("loss8", [T, 1], F32, kind="ExternalOutput").ap()
    return d


_CACHE = {}


def build_program():
    if "nc" in _CACHE:
        return _CACHE["nc"], _CACHE["io"]
    nc = bacc.Bacc("TRN2", target_bir_lowering=False, debug=False,
                   num_swdge_queues=4)
    io = _declare_io(nc)
    with tile.TileContext(nc) as tc:
        _build_kernel(tc, io)
    nc.compile()
    _CACHE["nc"] = nc
    _CACHE["io"] = io
    return nc, io


def host_prep_shared(embed_w, dom_w, fc1_w, fc1_b, fc2_w, fc2_b,
                     trans, start_s, end_s):
    f32 = np.float32
    bf16 = ml_dtypes.bfloat16
    fp8 = ml_dtypes.float8_e4m3
    p = np.arange(128)

    # fc1 x-part weights, DoubleRow layout (same as v1):
    # f1w8[kt, ch, s, p, m] = fc1_w[ch*128+m, 256*kt+2*p+s] * XSC
    w1x = np.asarray(fc1_w, f32)[:, :E] * XSC
    f1w8_arr = np.zeros((FKT, 2, 2, 128, 128), f32)
    for kt in range(FKT):
        for ch in range(2):
            for s_ in range(2):
                feat = 256 * kt + 2 * p + s_
                f1w8_arr[kt, ch, s_] = w1x[ch * 128:(ch + 1) * 128, feat].T
    f1w8_dram = np.ascontiguousarray(
        f1w8_arr.reshape(FKT * 2 * 2 * 128, 128)).astype(fp8)

    # host-computed Vd = dom_w @ W1d^T [10, 256], fp8 zero-padded to K=128 so
    # the lhsT gets fast-weight-load; the one-hot rhs carries the balancing
    # power-of-two scale so the product lands at XSC^2 like the fp8 x-part.
    Vd = np.asarray(dom_w, f32) @ np.asarray(fc1_w, f32)[:, E:].T
    amax = max(float(np.abs(Vd).max()), 1e-30)
    scl = float(2.0 ** np.floor(np.log2(224.0 / amax)))
    scl = max(min(scl, 2.0 ** 13), 2.0 ** -13)
    ocval = (XSC * XSC) / scl
    vdb = np.zeros((128, 2, 128), f32)  # [corp(pad), ch, m]
    for ch in range(2):
        vdb[:NCORP, ch, :] = Vd[:, ch * 128:(ch + 1) * 128] * scl
    vd8_dram = np.ascontiguousarray(vdb.reshape(128, 256)).astype(fp8)

    # shared bf16 pieces
    expt = np.exp(np.asarray(trans, np.float64)) / 8.0      # E' [j->k]
    estar = expt.T.astype(f32)                              # [k, j]
    out = {
        "emb8": np.ascontiguousarray(
            (np.asarray(embed_w, f32) * XSC).astype(fp8)),
        "f1w8": f1w8_dram,
        "vd8": vd8_dram,
        "_ocval": ocval,
        "f2wb": np.ascontiguousarray(
            np.asarray(fc2_w, f32).T.reshape(2, 128, T).transpose(1, 0, 2)
            .reshape(128, 2 * T).astype(bf16)),
        "_estar": estar,
        "_f1b2": np.asarray(fc1_b, f32).reshape(2, 128).T,  # [128, 2]
        "_estart_v": np.exp(np.asarray(start_s, np.float64)).astype(f32),
        "_eend_v": np.exp(np.asarray(end_s, np.float64)).astype(f32),
    }
    ones160 = np.zeros((1, 160), f32)
    ones160[0, :128] = 1.0
    ones160[0, 128:160] = np.tile(np.asarray(fc2_b, f32), 4)
    out["onesb"] = ones160.astype(bf16)
    # shift matrices for the chunk-combine tree
    shifts = []
    for off in (1, 2, 4, 8):
        shifts.append((p[:, None] == (p[None, :] + off) % 128).astype(f32))
    out["shf8"] = np.ascontiguousarray(
        np.stack(shifts, axis=1).reshape(128, 4 * 128).astype(bf16))
    return out


def host_prep_core(shared, words_sh, target_sh, corpus_sh, trans, start_s,
                   end_s):
    """Per-core blobs. words_sh [8,512], target_sh [8,512], corpus_sh [8]."""
    f32 = np.float32
    bf16 = ml_dtypes.bfloat16
    fp8 = ml_dtypes.float8_e4m3
    w = np.asarray(words_sh).astype(np.int64)       # [8, 512]
    tg = np.asarray(target_sh).astype(np.int64)
    cor = np.asarray(corpus_sh).astype(np.int64)
    trans = np.asarray(trans, f32)
    start_s = np.asarray(start_s, f32)
    end_s = np.asarray(end_s, f32)

    p = np.arange(128)
    bidx = p // K           # batch element of partition u
    cidx = p % K            # chunk of partition u
    # s for (u, l): s = cidx*32 + l
    lgrid = np.arange(L)[None, :]
    sgrid = cidx[:, None] * L + lgrid                # [128, 32]
    w_ul = w[bidx[:, None], sgrid]                   # [128, 32]
    t_ul = tg[bidx[:, None], sgrid]
    m_ul = (w_ul != 0).astype(f32)
    msc = m_ul * (sgrid > 0)

    # gather indices in tau order (tau = l*128 + u), wrapped per the ucode
    # gather convention: within each gather block, idx i at [i%16, col0+i//16],
    # replicated across the 8 groups of 16 partitions.
    perm = w.reshape(BSH, K, L).transpose(2, 0, 1).reshape(TOK)
    idx16 = np.zeros((128, TOK // 16), np.int16)
    for (l0, nl) in GROUPS:
        gt = nl * 128
        chunk = perm[l0 * 128:l0 * 128 + gt].astype(np.int16)
        t16 = chunk.reshape(gt // 16, 16).T          # idx i at [i%16, i//16]
        idx16[:, (l0 * 128) // 16:(l0 * 128 + gt) // 16] = np.tile(t16, (8, 1))

    ohcm = (np.arange(T)[None, None, :] == t_ul[:, :, None]).astype(f32) \
        * m_ul[:, :, None]                           # [128, 32, 8]

    c0 = (cidx == 0)
    m_b = (w != 0).astype(np.float64)                # [8, 512]
    msum1 = m_b[:, 1:].sum(1)
    goldtrans = (trans[tg[:, :-1], tg[:, 1:]] * m_b[:, 1:]).sum(1)
    last_idx = (m_b.sum(1).astype(int) - 1)
    score_se = start_s[tg[:, 0]] + end_s[tg[np.arange(BSH), last_idx]]
    gold = goldtrans + score_se
    hostc = np.where(c0, (msum1 * LOG8 - gold)[bidx], 0.0).astype(f32)
    c0m = np.where(c0, (1.0 - m_b[:, 0])[bidx], 0.0).astype(f32)
    ind0 = c0.astype(f32)

    fb32 = np.zeros((128, F32_W), f32)

    def put32(key, val):
        off, wd = F32_OFF[key]
        fb32[:, off:off + wd] = val

    put32("f1b2", shared["_f1b2"])
    put32("hostc", hostc[:, None])
    put32("c0m", c0m[:, None])
    put32("ind0", ind0[:, None])
    put32("indinv", (1.0 - ind0)[:, None])
    put32("g1", (np.arange(T)[None, :] == bidx[:, None]).astype(f32))

    bfb = np.zeros((128, BF_W), f32)

    def putbf(key, val):
        off, wd = BF_OFF[key]
        bfb[:, off:off + wd] = val.reshape(128, wd)

    putbf("mscb", msc)
    putbf("wneg", 1.0 - msc)
    putbf("ohcm", ohcm)
    putbf("estart", np.where(c0[:, None], shared["_estart_v"][None, :], 0.0))
    putbf("einitb", np.where(c0[:, None], 0.0,
                             np.eye(T, dtype=f32).reshape(1, T * T)))
    putbf("eendr", np.tile(shared["_eend_v"][None, :], (128, 1)))
    putbf("estar", np.tile(shared["_estar"].reshape(1, T * T), (128, 1)))

    # ocrb [128, 512]: onehot(corpus) * ocval (pow2, fp8-exact), K-padded
    corU = cor[bidx]                                  # [128]
    ocr = np.zeros((128, 512), f32)
    for k in range(NCORP):
        ocr[k] = np.tile((corU == k).astype(f32), 4) * shared["_ocval"]
    return {
        "idx16": idx16,
        "idx32": np.ascontiguousarray(w_ul[:, 0:4].astype(np.int32)),
        "bfb": np.ascontiguousarray(bfb.astype(bf16)),
        "fb32": np.ascontiguousarray(fb32),
        "ocr8": np.ascontiguousarray(ocr).astype(fp8),
    }


def make_in_maps(inputs):
    shared = host_prep_shared(
        inputs["embed_w"], inputs["dom_w"], inputs["fc1_w"], inputs["fc1_b"],
        inputs["fc2_w"], inputs["fc2_b"], inputs["trans"], inputs["start_s"],
        inputs["end_s"])
    words = np.asarray(inputs["words"]).astype(np.int64)
    target = np.asarray(inputs["target"]).astype(np.int64)
    corpus = np.asarray(inputs["corpus"]).astype(np.int64)
    in_maps = []
    for i in range(NC_N):
        per = host_prep_core(shared,
                             words[i * BSH:(i + 1) * BSH],
                             target[i * BSH:(i + 1) * BSH],
                             corpus[i * BSH:(i + 1) * BSH],
                             inputs["trans"], inputs["start_s"],
                             inputs["end_s"])
        full = {**shared, **per}
        in_maps.append({k: full[k] for k in INPUT_KEYS})
    return in_maps


LAST_RESULTS = None


def _ensure_axon_hooks_shim():
    try:
        import antenv.axon_hooks  # noqa: F401
        return
    except ImportError:
        pass
    try:
        import types
        import antenv
        mod = types.ModuleType("antenv.axon_hooks")
        _state = {"hook": None}
        mod.set_axon_ntff_profile_hook = \
            lambda h: _state.__setitem__("hook", h)
        mod.get_axon_ntff_profile_hook = lambda: _state["hook"]
        sys.modules["antenv.axon_hooks"] = mod
        antenv.axon_hooks = mod
    except Exception:
        pass


def kernel(**inputs):
    global LAST_RESULTS
    from concourse.bass_utils import run_bass_kernel_spmd

    _ensure_axon_hooks_shim()
    nc, _ = build_program()
    in_maps = make_in_maps(inputs)
    res = run_bass_kernel_spmd(nc, in_maps, list(range(NC_N)))
    LAST_RESULTS = res
    out = np.concatenate(
        [np.asarray(res.results[i]["loss8"], np.float32).reshape(BSH)
         for i in range(NC_N)])
    return out
